# revision 14
# baseline (speedup 1.0000x reference)
"""Trainium2 Bass kernel for MindSpeed TE grouped linear (MoE grouped GEMM).

Computes, for E=64 experts with row splits m_splits (sum = 32768):
    y[rows_e, :] = x[rows_e, :] @ W[e].T        W[e]: [1408, 2048]

Strategy: pure expert-parallel over 8 NeuronCores — core c owns experts
[8c, 8c+8) and their (contiguous) token rows. No collectives; gather is a
host-side concat. Host pre-transposes both operands into K-major layouts
whose DMA slices are large contiguous runs per partition:
    xT [P, MT, KO, 128]  (8 KB/partition per 2-m-tile chunk)
    wT [E, P, KO, OUT]   (5.6 KB/partition per eighth-K granule)
Both operands and the output travel as fp16 (halves HBM traffic vs fp32;
~2.5e-4 rel err). Per m-tile the kernel holds the x chunk stationary and
streams the expert's full 1408 output columns through 3 PSUM banks,
accumulating over the 16 K-chunks, then downcasts PSUM->fp16 and stores
one full-width y row-block per m-tile.
"""

import math

import numpy as np

import concourse.mybir as mybir
import concourse.tile as tile
from concourse import bacc
from concourse.bass_utils import run_bass_kernel_spmd

N_CORES = 8
P = 128
IN_SIZE = 2048
OUT_SIZE = 1408
KO = IN_SIZE // P  # 16 contraction subtiles
KQ = 4  # W arrives in quarter-K granules (1.44 MB) for pipelining
NQ = KO // KQ

# PSUM n-tiles: one bank is 512 fp32 per partition
N_TILES = [(0, 512), (512, 512), (1024, 384)]

_nc_cache: dict = {}


def _build(pattern: tuple) -> "bacc.Bacc":
    """One SPMD program: `pattern` = per-expert (padded) token counts for the
    8 local experts of a core; identical across cores."""
    T = sum(pattern)
    E_loc = len(pattern)
    MT = T // P
    nc = bacc.Bacc(None, target_bir_lowering=False, name="grouped_linear")
    xT = nc.dram_tensor(
        "xT", [P, MT, KO, P], mybir.dt.float16, kind="ExternalInput"
    )
    wT = nc.dram_tensor(
        "wT", [E_loc, P, KO, OUT_SIZE], mybir.dt.float16, kind="ExternalInput"
    )
    y = nc.dram_tensor("y", [T, OUT_SIZE], mybir.dt.float16, kind="ExternalOutput")

    segs = []  # (expert, first m-tile, m-tile count)
    mt0 = 0
    for e in range(E_loc):
        mts = pattern[e] // P
        if mts:
            segs.append((e, mt0, mts))
        mt0 += mts

    with tile.TileContext(nc) as tc:
        with (
            tc.tile_pool(name="xp", bufs=5) as xpool,
            tc.tile_pool(name="wp", bufs=10) as wpool,
            tc.tile_pool(name="op", bufs=4) as opool,
            tc.tile_pool(name="ps", bufs=6, space="PSUM") as pspool,
            tc.tile_pool(name="wm", bufs=1) as wmpool,
            tc.tile_pool(name="wmp", bufs=1, space="PSUM") as wmpspool,
        ):
            # PE warmup: throwaway matmuls on zeroed SBUF into a scratch
            # PSUM bank. They run during the DMA ramp (no data deps), keep
            # the HAM clock gate at K=8/8 so the first real matmuls start
            # at 2.4 GHz instead of half clock.
            warm_x = wmpool.tile([P, P], mybir.dt.float16, tag="wx", name="warm_x")
            warm_w = wmpool.tile([P, 512], mybir.dt.float16, tag="ww", name="warm_w")
            warm_ps = wmpspool.tile([P, 512], mybir.dt.float32, tag="wp", name="warm_ps")
            nc.vector.memset(warm_x, 0)
            nc.vector.memset(warm_w, 0)
            for _ in range(16):
                nc.tensor.matmul(warm_ps, warm_x, warm_w, start=True, stop=True)

            for si, (e, mt0, mts) in enumerate(segs):
                nchunks = -(-mts // 2)

                def load_x(c, split=False):
                    csz = min(2, mts - c * 2)
                    x_c = xpool.tile(
                        [P, 2, KO, P], mybir.dt.float16, tag="x", name="x_c"
                    )
                    if split and csz == 2:
                        # ramp: land the first m-tile's x half as early as
                        # possible so LDWEIGHTS can start sooner
                        nc.sync.dma_start(x_c[:, :1], xT[:, mt0 : mt0 + 1])
                        nc.sync.dma_start(
                            x_c[:, 1:2], xT[:, mt0 + 1 : mt0 + 2]
                        )
                    else:
                        nc.sync.dma_start(
                            x_c[:, :csz], xT[:, mt0 + c * 2 : mt0 + c * 2 + csz]
                        )
                    return x_c

                # First x chunk before the W granules so the opening
                # matmul's inputs land with minimal ramp; the rest after.
                x_cs = [load_x(0, split=(si == 0))]
                w_qs = []
                for q in range(NQ):
                    w_q = wpool.tile(
                        [P, KQ, OUT_SIZE], mybir.dt.float16, tag="w", name="w_q"
                    )
                    nc.sync.dma_start(
                        w_q, wT[e, :, q * KQ : (q + 1) * KQ, :]
                    )
                    w_qs.append(w_q)
                for c in range(1, nchunks):
                    x_cs.append(load_x(c))

                def flush(mt, ps_ts):
                    o_t = opool.tile(
                        [P, OUT_SIZE], mybir.dt.float16, tag="o", name="o_t"
                    )
                    for ni, (n0, nsz) in enumerate(N_TILES):
                        nc.vector.tensor_copy(
                            o_t[:, n0 : n0 + nsz], ps_ts[ni][:, :nsz]
                        )
                    nc.scalar.dma_start(
                        y[(mt0 + mt) * P : (mt0 + mt + 1) * P, :], o_t
                    )

                def mm(ps_ts, x_c, j, ko):
                    q, k = divmod(ko, KQ)
                    lhsT = x_c[:, j, ko, :]
                    for ni, (n0, nsz) in enumerate(N_TILES):
                        nc.tensor.matmul(
                            ps_ts[ni][:, :nsz],
                            lhsT,
                            w_qs[q][:, k, n0 : n0 + nsz],
                            start=(ko == 0),
                            stop=(ko == KO - 1),
                        )

                def ps_alloc():
                    return [
                        pspool.tile([P, 512], mybir.dt.float32, tag="ps", name="ps_t")
                        for _ in N_TILES
                    ]

                if si == 0 and mts == 2:
                    # Ramp: quarter-K-major over both m-tiles (6 live PSUM
                    # banks) so each W granule is fully consumed before the
                    # next is needed — matmuls start after ~2.4 MB of DMA
                    # instead of stalling on the whole 5.8 MB expert.
                    ps_pair = [ps_alloc(), ps_alloc()]
                    for q in range(NQ):
                        for mt in range(2):
                            for k in range(KQ):
                                mm(ps_pair[mt], x_cs[0], mt, q * KQ + k)
                    for mt in range(2):
                        flush(mt, ps_pair[mt])
                else:
                    for mt in range(mts):
                        ps_ts = ps_alloc()
                        for ko in range(KO):
                            mm(ps_ts, x_cs[mt // 2], mt % 2, ko)
                        flush(mt, ps_ts)
    nc.compile()
    return nc


def _get_nc(pattern: tuple) -> "bacc.Bacc":
    nc = _nc_cache.get(pattern)
    if nc is None:
        nc = _build(pattern)
        _nc_cache[pattern] = nc
    return nc


def _plan(splits: np.ndarray):
    """Choose a per-core expert-size pattern (identical across cores, sizes
    multiples of 128). Returns (padded_pattern, per-core list of per-expert
    actual sizes)."""
    E = len(splits)
    epc = E // N_CORES
    per_core = [tuple(int(s) for s in splits[c * epc : (c + 1) * epc]) for c in range(N_CORES)]
    uniform = all(p == per_core[0] for p in per_core)
    if uniform:
        padded = tuple(128 * math.ceil(s / 128) for s in per_core[0])
    else:
        m_pad = 128 * math.ceil(int(max(splits.max(), 1)) / 128)
        padded = (m_pad,) * epc
    return padded, per_core


def kernel(x: np.ndarray, W: np.ndarray, m_splits: np.ndarray, _profile=None) -> np.ndarray:
    x = np.ascontiguousarray(np.asarray(x), dtype=np.float32)
    W = np.ascontiguousarray(np.asarray(W), dtype=np.float32)
    raw = np.asarray(m_splits).astype(np.int64)
    E = raw.shape[0]
    assert E % N_CORES == 0 and W.shape[0] == E
    epc = E // N_CORES
    # Mirror the reference's python-slice semantics: x[offs[e]:offs[e+1]]
    # clips to the array bounds, so effective sizes come from clipped offsets.
    raw_offs = np.concatenate([[0], np.cumsum(np.maximum(raw, 0))])
    lo = np.minimum(raw_offs[:-1], x.shape[0])
    hi = np.minimum(raw_offs[1:], x.shape[0])
    splits = np.maximum(hi - lo, 0)
    offs = np.concatenate([[0], np.cumsum(splits)])
    total = int(offs[-1])

    padded, per_core = _plan(splits)
    pofs = np.concatenate([[0], np.cumsum(padded)])
    T_pad = int(pofs[-1])

    nc = _get_nc(padded)

    in_maps = []
    for c in range(N_CORES):
        if tuple(padded) == per_core[c]:
            xs = x[lo[c * epc] : hi[(c + 1) * epc - 1]]
        else:
            xs = np.zeros((T_pad, IN_SIZE), dtype=np.float32)
            for e in range(epc):
                g = c * epc + e
                xs[pofs[e] : pofs[e] + splits[g]] = x[lo[g] : hi[g]]
        xTc = (
            xs.reshape(T_pad // P, P, KO, P)
            .transpose(3, 0, 2, 1)
            .astype(np.float16)
        )
        wTc = (
            W[c * epc : (c + 1) * epc]
            .reshape(epc, OUT_SIZE, KO, P)
            .transpose(0, 3, 2, 1)
            .astype(np.float16)
        )
        in_maps.append(
            {"xT": np.ascontiguousarray(xTc), "wT": np.ascontiguousarray(wTc)}
        )

    kwargs = dict(_profile) if _profile else {}
    res = run_bass_kernel_spmd(nc, in_maps, core_ids=list(range(N_CORES)), **kwargs)
    if _profile is not None:
        _profile["result"] = res

    out = np.empty((total, OUT_SIZE), dtype=np.float32)
    for c in range(N_CORES):
        yc = res.results[c]["y"].astype(np.float32)
        for e in range(epc):
            g = c * epc + e
            out[offs[g] : offs[g + 1]] = yc[pofs[e] : pofs[e] + splits[g]]
    return out


# revision 16
# speedup vs baseline: 1.0094x; 1.0094x over previous
"""Trainium2 Bass kernel for MindSpeed TE grouped linear (MoE grouped GEMM).

Computes, for E=64 experts with row splits m_splits (sum = 32768):
    y[rows_e, :] = x[rows_e, :] @ W[e].T        W[e]: [1408, 2048]

Strategy: pure expert-parallel over 8 NeuronCores — core c owns experts
[8c, 8c+8) and their (contiguous) token rows. No collectives; gather is a
host-side concat. Host pre-transposes both operands into K-major layouts
whose DMA slices are large contiguous runs per partition:
    xT [P, MT, KO, 128]  (8 KB/partition per 2-m-tile chunk)
    wT [E, P, KO, OUT]   (5.6 KB/partition per eighth-K granule)
Both operands and the output travel as fp16 (halves HBM traffic vs fp32;
~2.5e-4 rel err). Per m-tile the kernel holds the x chunk stationary and
streams the expert's full 1408 output columns through 3 PSUM banks,
accumulating over the 16 K-chunks, then downcasts PSUM->fp16 and stores
one full-width y row-block per m-tile.
"""

import math

import numpy as np

import concourse.mybir as mybir
import concourse.tile as tile
from concourse import bacc
from concourse.bass_utils import run_bass_kernel_spmd

N_CORES = 8
P = 128
IN_SIZE = 2048
OUT_SIZE = 1408
KO = IN_SIZE // P  # 16 contraction subtiles
KQ = 4  # W arrives in quarter-K granules (1.44 MB) for pipelining
NQ = KO // KQ

# PSUM n-tiles: one bank is 512 fp32 per partition
N_TILES = [(0, 512), (512, 512), (1024, 384)]

_nc_cache: dict = {}


def _build(pattern: tuple) -> "bacc.Bacc":
    """One SPMD program: `pattern` = per-expert (padded) token counts for the
    8 local experts of a core; identical across cores."""
    T = sum(pattern)
    E_loc = len(pattern)
    MT = T // P
    nc = bacc.Bacc(None, target_bir_lowering=False, name="grouped_linear")
    xT = nc.dram_tensor(
        "xT", [P, MT, KO, P], mybir.dt.float16, kind="ExternalInput"
    )
    wT = nc.dram_tensor(
        "wT", [E_loc, P, KO, OUT_SIZE], mybir.dt.float16, kind="ExternalInput"
    )
    y = nc.dram_tensor("y", [T, OUT_SIZE], mybir.dt.float16, kind="ExternalOutput")

    segs = []  # (expert, first m-tile, m-tile count)
    mt0 = 0
    for e in range(E_loc):
        mts = pattern[e] // P
        if mts:
            segs.append((e, mt0, mts))
        mt0 += mts

    with tile.TileContext(nc) as tc:
        with (
            tc.tile_pool(name="xp", bufs=5) as xpool,
            tc.tile_pool(name="wp", bufs=10) as wpool,
            tc.tile_pool(name="op", bufs=4) as opool,
            tc.tile_pool(name="ps", bufs=6, space="PSUM") as pspool,
        ):
            for si, (e, mt0, mts) in enumerate(segs):
                nchunks = -(-mts // 2)

                def load_x(c):
                    csz = min(2, mts - c * 2)
                    x_c = xpool.tile(
                        [P, 2, KO, P], mybir.dt.float16, tag="x", name="x_c"
                    )
                    nc.sync.dma_start(
                        x_c[:, :csz], xT[:, mt0 + c * 2 : mt0 + c * 2 + csz]
                    )
                    return x_c

                def load_w(q):
                    w_q = wpool.tile(
                        [P, KQ, OUT_SIZE], mybir.dt.float16, tag="w", name="w_q"
                    )
                    nc.sync.dma_start(w_q, wT[e, :, q * KQ : (q + 1) * KQ, :])
                    return w_q

                # First x chunk before the W granules so the opening
                # matmul's inputs land with minimal ramp; the rest after.
                if si == 0 and mts >= 2:
                    # interleave the opening issues so the first matmul's
                    # inputs (x m-tile 0 + W granule 0) are on the wire first
                    x_c = xpool.tile(
                        [P, 2, KO, P], mybir.dt.float16, tag="x", name="x_c"
                    )
                    nc.sync.dma_start(x_c[:, :1], xT[:, mt0 : mt0 + 1])
                    x_cs = [x_c]
                    w_qs = [load_w(0)]
                    nc.sync.dma_start(x_c[:, 1:2], xT[:, mt0 + 1 : mt0 + 2])
                    w_qs += [load_w(q) for q in range(1, NQ)]
                else:
                    x_cs = [load_x(0)]
                    w_qs = [load_w(q) for q in range(NQ)]
                for c in range(1, nchunks):
                    x_cs.append(load_x(c))

                def flush(mt, ps_ts):
                    o_t = opool.tile(
                        [P, OUT_SIZE], mybir.dt.float16, tag="o", name="o_t"
                    )
                    for ni, (n0, nsz) in enumerate(N_TILES):
                        nc.vector.tensor_copy(
                            o_t[:, n0 : n0 + nsz], ps_ts[ni][:, :nsz]
                        )
                    nc.scalar.dma_start(
                        y[(mt0 + mt) * P : (mt0 + mt + 1) * P, :], o_t
                    )

                def mm(ps_ts, x_c, j, ko):
                    q, k = divmod(ko, KQ)
                    lhsT = x_c[:, j, ko, :]
                    for ni, (n0, nsz) in enumerate(N_TILES):
                        nc.tensor.matmul(
                            ps_ts[ni][:, :nsz],
                            lhsT,
                            w_qs[q][:, k, n0 : n0 + nsz],
                            start=(ko == 0),
                            stop=(ko == KO - 1),
                        )

                def ps_alloc():
                    return [
                        pspool.tile([P, 512], mybir.dt.float32, tag="ps", name="ps_t")
                        for _ in N_TILES
                    ]

                if si == 0 and mts == 2:
                    # Ramp: quarter-K-major over both m-tiles (6 live PSUM
                    # banks) so each W granule is fully consumed before the
                    # next is needed — matmuls start after ~2.4 MB of DMA
                    # instead of stalling on the whole 5.8 MB expert.
                    ps_pair = [ps_alloc(), ps_alloc()]
                    for q in range(NQ):
                        for mt in range(2):
                            for k in range(KQ):
                                mm(ps_pair[mt], x_cs[0], mt, q * KQ + k)
                    for mt in range(2):
                        flush(mt, ps_pair[mt])
                else:
                    for mt in range(mts):
                        ps_ts = ps_alloc()
                        for ko in range(KO):
                            mm(ps_ts, x_cs[mt // 2], mt % 2, ko)
                        flush(mt, ps_ts)
    nc.compile()
    return nc


def _get_nc(pattern: tuple) -> "bacc.Bacc":
    nc = _nc_cache.get(pattern)
    if nc is None:
        nc = _build(pattern)
        _nc_cache[pattern] = nc
    return nc


def _plan(splits: np.ndarray):
    """Choose a per-core expert-size pattern (identical across cores, sizes
    multiples of 128). Returns (padded_pattern, per-core list of per-expert
    actual sizes)."""
    E = len(splits)
    epc = E // N_CORES
    per_core = [tuple(int(s) for s in splits[c * epc : (c + 1) * epc]) for c in range(N_CORES)]
    uniform = all(p == per_core[0] for p in per_core)
    if uniform:
        padded = tuple(128 * math.ceil(s / 128) for s in per_core[0])
    else:
        m_pad = 128 * math.ceil(int(max(splits.max(), 1)) / 128)
        padded = (m_pad,) * epc
    return padded, per_core


def kernel(x: np.ndarray, W: np.ndarray, m_splits: np.ndarray, _profile=None) -> np.ndarray:
    x = np.ascontiguousarray(np.asarray(x), dtype=np.float32)
    W = np.ascontiguousarray(np.asarray(W), dtype=np.float32)
    raw = np.asarray(m_splits).astype(np.int64)
    E = raw.shape[0]
    assert E % N_CORES == 0 and W.shape[0] == E
    epc = E // N_CORES
    # Mirror the reference's python-slice semantics: x[offs[e]:offs[e+1]]
    # clips to the array bounds, so effective sizes come from clipped offsets.
    raw_offs = np.concatenate([[0], np.cumsum(np.maximum(raw, 0))])
    lo = np.minimum(raw_offs[:-1], x.shape[0])
    hi = np.minimum(raw_offs[1:], x.shape[0])
    splits = np.maximum(hi - lo, 0)
    offs = np.concatenate([[0], np.cumsum(splits)])
    total = int(offs[-1])

    padded, per_core = _plan(splits)
    pofs = np.concatenate([[0], np.cumsum(padded)])
    T_pad = int(pofs[-1])

    nc = _get_nc(padded)

    in_maps = []
    for c in range(N_CORES):
        if tuple(padded) == per_core[c]:
            xs = x[lo[c * epc] : hi[(c + 1) * epc - 1]]
        else:
            xs = np.zeros((T_pad, IN_SIZE), dtype=np.float32)
            for e in range(epc):
                g = c * epc + e
                xs[pofs[e] : pofs[e] + splits[g]] = x[lo[g] : hi[g]]
        xTc = (
            xs.reshape(T_pad // P, P, KO, P)
            .transpose(3, 0, 2, 1)
            .astype(np.float16)
        )
        wTc = (
            W[c * epc : (c + 1) * epc]
            .reshape(epc, OUT_SIZE, KO, P)
            .transpose(0, 3, 2, 1)
            .astype(np.float16)
        )
        in_maps.append(
            {"xT": np.ascontiguousarray(xTc), "wT": np.ascontiguousarray(wTc)}
        )

    kwargs = dict(_profile) if _profile else {}
    res = run_bass_kernel_spmd(nc, in_maps, core_ids=list(range(N_CORES)), **kwargs)
    if _profile is not None:
        _profile["result"] = res

    out = np.empty((total, OUT_SIZE), dtype=np.float32)
    for c in range(N_CORES):
        yc = res.results[c]["y"].astype(np.float32)
        for e in range(epc):
            g = c * epc + e
            out[offs[g] : offs[g + 1]] = yc[pofs[e] : pofs[e] + splits[g]]
    return out
